# revision 47
# baseline (speedup 1.0000x reference)
"""Trainium2 Bass kernel for nn_DeformAtten1D (B=4, S=4096, D=1024, H=16, G=4, K=3).

Math: the reference's grid-sample degenerates (iy = (S-1)/2 fixed, width dim = 1), so
x_sampled = feat_c (outer) wx  is rank-1 per (batch, group).  Propagating that structure
collapses every large GEMM:

  offset[g,s] = sum_k a_{g,k} . x[s+k-1,:]      (a_{g,k} = Wq_g^T @ w_eff_k, weight-only)
  wx[g,s]     = 1 - |tanh(offset)*K/(S-1) + s/(S-1) - 0.5|      (clip provably inactive)
  xWx5T       = [wx;1] @ x                      [5, D]   (the only s-reduction over x)
  qaT         = scale * xWx5T @ Wq^T            [5, D]
  kbT/vbT     = [0.5*featBD^T @ W^T ; bias]     [5, D]
  scT_h       = kbT_h^T @ qaT_h   (scores transposed; exp safe without max-sub
                since |scores| < 5 for this operator scale)
  U_h         = exp(scT_h) @ [vb_h | 1]  ->  Astack_h = U[:,0:5] / U[:,5]
  MT          = Astack^T @ Wo^T ;  Mc6 = [MT[0:4]; MT[4]+bo; Wo@1]
  y[s,:]      = [wx[:,s]; 1; bt[s]]^T @ Mc6     (bias_table term: attn row-sums == 1)

Sharding: core c -> (batch c//2, sequence half c%2), S_SH=2048; attention heads are
additionally split across the pair (even core: heads 0-7, odd: 8-15) so each core
reads only half of each projection weight.  Cross-core data: two pairwise AllReduces
of [5,1024] partials (xWx5T after phase A, MT after phase B).

Perf notes (122.6us -> ~96.5us on the TimelineSim cost model):
 - DMA count collapsed ~110 -> ~30: inputs are host-packed partition-major
   ([128, tiles, cols]) and loaded as a handful of large transfers (each DMA
   costs ~625ns serialized HWDGE issue + ~900ns completion latency in the
   model, and all transfers serialize on one DMA_ENGINES resource).
 - x (both layouts) and all four projection-weight halves load as bf16,
   halving input HBM bytes (end-to-end error validated ~4e-3 vs 2e-2 budget).
 - xT/xn stream in interleaved 512-column chunks so phase A starts after ~3us
   and the xWx accumulation chases the arriving data.
 - Phase A is emitted chunk-pipelined and stage-split (in-order engine queues:
   never park a ready matmul behind a stalled one); the wx middle is batched
   [128, w, 4] to amortize the ~220-cycle per-op engine access overhead.
 - Attention avoids per-head softmax chains: scores are built transposed
   (lhsT=kb, rhs=qa), exp needs no max-subtraction (|scores| < 5), the row-sum
   rides as a ones-column through the value matmul, and the normalization is a
   per-partition reciprocal+scale.  kb/vb/vb6 are emitted before the AR1
   readback to fill the collective's latency window.
 - Phase C writes y through ramped store groups (1,1,2,3,3,3,3 tiles) so the
   first store issues early while later groups amortize DMA issue cost.
Hardware runtime pitfalls baked in: PE cannot read lhsT from partition base
32/64 (runtime crash) -- the k-shift rebase uses DVE cross-partition-base
copies instead; GPSIMD cannot touch PSUM; DVE ops may read at most one PSUM
operand; f32r memsets and strided memsets fail the ISA check.
"""

import numpy as np
import ml_dtypes

B, S, D, H, G, K = 4, 4096, 1024, 16, 4, 3
DG, DH = D // G, D // H
NCORES = 8
SCALE = D ** (-0.5)
H_LOC = H // 2          # heads per core (pair-split)
DH_LOC = H_LOC * DH     # 512 channel columns per core

_CACHE = {}


def _build_bass(s_sh: int, offconst: float, sim_no_cc: bool = False):
    from contextlib import ExitStack
    import concourse.bass as bass
    import concourse.mybir as mybir
    import concourse.tile as tile
    from concourse import bacc
    from concourse.masks import make_identity

    fp32 = mybir.dt.float32
    f32r = mybir.dt.float32r
    bf16 = mybir.dt.bfloat16
    AF = mybir.ActivationFunctionType
    ALU = mybir.AluOpType

    n_st = s_sh // 128          # s-tiles (16)
    n_ch = s_sh // 512          # 512-wide chunks (4)
    n_dt = D // 128             # d-tiles (8)
    n_dt_h = n_dt // 2          # d-tiles of this core's head half (4)
    W = s_sh + 2                # halo width (2050)

    nc = bacc.Bacc(None, num_devices=NCORES)

    # --- host-packed DRAM inputs (partition-major: [128, tiles, cols]) ---
    xT_p = nc.declare_dram_parameter("xT_p", [128, n_dt, W], bf16, isOutput=False)
    xn_p = nc.declare_dram_parameter("xn_p", [128, n_st, D], bf16, isOutput=False)
    ab_p = nc.declare_dram_parameter("ab_p", [128, n_dt, 100], bf16, isOutput=False)
    Wq_p = nc.declare_dram_parameter("Wq_p", [128, n_dt, DH_LOC], bf16, isOutput=False)
    Wk_p = nc.declare_dram_parameter("Wk_p", [128, n_dt, DH_LOC], bf16, isOutput=False)
    Wv_p = nc.declare_dram_parameter("Wv_p", [128, n_dt, DH_LOC], bf16, isOutput=False)
    Wo_p = nc.declare_dram_parameter("Wo_p", [128, n_dt_h, D], bf16, isOutput=False)
    # sm1r: [bt(2048) | bk(512) | w1(1024) | ones(s_sh)] (f32r destinations)
    sm1r = nc.declare_dram_parameter("sm1r", [1, 2 * s_sh + 1536], f32r,
                                     isOutput=False)
    # sm1f: [bv(512) | bo(1024)] on partition 0 (fp32 destinations)
    sm1f = nc.declare_dram_parameter("sm1f", [1, 1536], fp32, isOutput=False)
    sm2 = nc.declare_dram_parameter("sm2", [128, n_st, 4], fp32, isOutput=False)
    y_p = nc.declare_dram_parameter("y", [128, n_st, D], fp32, isOutput=True)

    with tile.TileContext(nc) as tc, ExitStack() as ctx:
        P = ctx.enter_context(tc.tile_pool(name="persist", bufs=1))
        small = ctx.enter_context(tc.tile_pool(name="small", bufs=24))
        ypool = ctx.enter_context(tc.tile_pool(name="ypool", bufs=2))
        ps_a = ctx.enter_context(tc.tile_pool(name="ps_a", bufs=3, space="PSUM"))
        ps_fix = ctx.enter_context(tc.tile_pool(name="ps_fix", bufs=1, space="PSUM"))
        ps_pre = ctx.enter_context(tc.tile_pool(name="ps_pre", bufs=2, space="PSUM"))
        ps_tiny = ctx.enter_context(tc.tile_pool(name="ps_tiny", bufs=2, space="PSUM"))
        dram = ctx.enter_context(tc.tile_pool(name="dram", bufs=1, space="DRAM"))

        def pt(shape, tag, dtype=fp32):
            return P.tile(shape, dtype, tag=tag, name=tag)

        # ---------- constants / input DMAs (issue order == DMA queue order) ----
        ident = pt([128, 128], "ident")
        make_identity(nc, ident)

        # abuf first (preT lhsT), then xT in 4 chunks so the preT accumulation
        # starts after ~3us instead of waiting for one monolithic 12us DMA.
        abuf = pt([128, n_dt, 100], "abuf", bf16)   # [:, :, 0:96]=aoff, 96:100=featBD
        nc.sync.dma_start(abuf, ab_p[:, :, :])

        base_sb = pt([128, n_st, 4], "base")
        nc.sync.dma_start(base_sb, sm2[:, :, :])

        wx6T = pt([6, s_sh], "wx6T", f32r)          # rows 0-3 wx, 4 ones, 5 bias_table
        nc.sync.dma_start(wx6T[5:6, :], sm1r[:, 0:s_sh])
        nc.sync.dma_start(wx6T[4:5, :],
                          sm1r[:, s_sh + 1536:2 * s_sh + 1536])

        # xT and xn interleaved in COLUMN chunks: pre-chunk c (and the wx
        # tiles + xWx accumulation behind it) unblocks after ~3us*(c+1)
        # instead of waiting for the full 12us xT transfer.
        xT_sb = pt([128, n_dt, W], "xT", bf16)
        xn_sb = pt([128, n_st, D], "xn", bf16)
        kbT = pt([5, DH_LOC], "kbT", f32r)
        vbT = pt([5, DH_LOC], "vbT")
        Mc6 = pt([6, D], "Mc6", f32r)
        bo5 = pt([5, D], "bo5")
        nc.vector.memset(bo5, 0.0)

        xt_cuts = (0, 512, 1024, 1536, W)
        for q in range(4):
            nc.sync.dma_start(xT_sb[:, :, xt_cuts[q]:xt_cuts[q + 1]],
                              xT_p[:, :, xt_cuts[q]:xt_cuts[q + 1]])
            nc.sync.dma_start(xn_sb[:, 4 * q:4 * q + 4, :],
                              xn_p[:, 4 * q:4 * q + 4, :])
            if q == 0:
                # bias rows slot in behind the first x chunks; needed late
                nc.sync.dma_start(kbT[4:5, :], sm1r[:, s_sh:s_sh + 512])
                nc.sync.dma_start(vbT[4:5, :], sm1f[:, 0:512])
                nc.sync.dma_start(Mc6[5:6, :], sm1r[:, s_sh + 512:s_sh + 1536])
                nc.sync.dma_start(bo5[4:5, :], sm1f[:, 512:1536])

        Wk_sb = pt([128, n_dt, DH_LOC], "Wk", bf16)
        nc.sync.dma_start(Wk_sb, Wk_p[:, :, :])
        Wv_sb = pt([128, n_dt, DH_LOC], "Wv", bf16)
        nc.sync.dma_start(Wv_sb, Wv_p[:, :, :])
        Wq_sb = pt([128, n_dt, DH_LOC], "Wq", bf16)
        nc.sync.dma_start(Wq_sb, Wq_p[:, :, :])
        Wo_sb = pt([128, n_dt_h, D], "Wo", bf16)
        nc.sync.dma_start(Wo_sb, Wo_p[:, :, :])

        # ---------- phase A: preT[32k+g, w] = sum_d a_{g,k}[d] * xT[d, w] -------
        # The k=0/1/2 blocks live at partition bases 0/32/64; the +1/+2 column
        # shifts of the depthwise conv are folded into per-s-tile PE transposes
        # (three [4,128+shift] -> [128,4] transposes into one PSUM tile), so no
        # mid-phase DMAs enter the (FIFO) DMA queue behind the weight loads.
        preT = pt([96, W], "preT")
        preS1 = pt([4, s_sh], "preS1")
        preS2 = pt([4, s_sh], "preS2")
        wx5_all = pt([128, n_st, 5], "wx5", bf16)
        wxa_all = pt([128, n_st, 4], "wxa")
        # contiguous full-tile memset (strided memset fails the ISA check);
        # columns 0-3 are overwritten per tile, column 4 stays == 1
        nc.vector.memset(wx5_all, 1.0)
        tt_all = ps_fix.tile([128, n_st, 12], fp32, tag="tt", name="tt_all")
        xwx_lo = ps_a.tile([5, 512], fp32, tag="a", name="xwx_lo")
        xwx_hi = ps_a.tile([5, 512], fp32, tag="a", name="xwx_hi")
        xwx_sb = pt([5, D], "xwx_sb")

        def pre_chunk(c):
            w0 = c * 512
            nw = 512 if c < n_ch else 2        # tail: last 2 halo columns
            pre_ps = ps_pre.tile([96, 512], fp32, tag="pre", name="pre_ps")
            for ct in range(n_dt):
                nc.tensor.matmul(
                    pre_ps[:, 0:nw], lhsT=abuf[:, ct, 0:96],
                    rhs=xT_sb[:, ct, w0: w0 + nw],
                    start=(ct == 0), stop=(ct == n_dt - 1))
            nc.scalar.activation(preT[:, w0:w0 + nw], pre_ps[:, 0:nw], AF.Copy)

        def rebase(c):
            # DVE cross-partition-base copies fold the +1/+2 column shifts of
            # the k=1/k=2 blocks down to partitions 0-3 (PE cannot read lhsT
            # from partition base 32/64 -- that crashes the runtime)
            w0 = c * 512
            nc.vector.tensor_copy(preS1[:, w0:w0 + 512],
                                  preT[32:36, w0 + 1:w0 + 513])
            nc.vector.tensor_copy(preS2[:, w0:w0 + 512],
                                  preT[64:68, w0 + 2:w0 + 514])

        def wx_s1(st):
            # stage 1 (PE): three k-block transposes into tt_all[st]
            c0 = st * 128
            for k, srcb in enumerate((preT, preS1, preS2)):
                nc.tensor.transpose(
                    tt_all[:, st, 4 * k:4 * k + 4],
                    srcb[0:4, c0:c0 + 128], ident[0:4, 0:4])

        def wx_s2(t0, t1):
            # stage 2, batched over tiles [t0, t1): shift-add, tanh,
            # |K/(S-1)*x + base|, 1-x.  Batching amortizes the ~220-cycle
            # per-op SBUF access overhead that dominated [128,4]-shaped ops.
            w = t1 - t0
            # hw limit: at most one PSUM operand per instruction -- stage the
            # k=0 block through SBUF, then chain single-PSUM adds
            c0 = small.tile([128, w, 4], fp32, name="c0")
            nc.vector.tensor_copy(c0, tt_all[:, t0:t1, 0:4])
            s01 = small.tile([128, w, 4], fp32, name="s01")
            nc.vector.tensor_add(out=s01, in0=c0, in1=tt_all[:, t0:t1, 4:8])
            s012 = small.tile([128, w, 4], fp32, name="s012")
            nc.vector.tensor_add(out=s012, in0=s01, in1=tt_all[:, t0:t1, 8:12])
            tgt = small.tile([128, w, 4], fp32, name="tgt")
            nc.scalar.activation(tgt, s012, AF.Tanh, bias=float(offconst))
            u = small.tile([128, w, 4], fp32, name="u")
            nc.vector.scalar_tensor_tensor(
                out=u, in0=tgt, scalar=float(K / (S - 1)),
                in1=base_sb[:, t0:t1, :], op0=ALU.mult, op1=ALU.add)
            a = wxa_all[:, t0:t1, :]
            nc.scalar.activation(a, u, AF.Abs)
            nc.scalar.activation(wx5_all[:, t0:t1, 0:4], a, AF.Copy,
                                 scale=-1.0, bias=1.0)

        def wx_s3(st):
            # stage 3 (PE+DVE): transpose |.| back, 1-x on the way into wx6T
            # rows 0-3 (row 4 is DMA-loaded ones); xWx accumulation
            wT_ps = ps_tiny.tile([4, 128], fp32, tag="tiny", name="wT_ps")
            nc.tensor.transpose(wT_ps[0:4, 0:128], wxa_all[:, st, :], ident)
            nc.vector.tensor_scalar(
                out=wx6T[0:4, st * 128:(st + 1) * 128], in0=wT_ps[0:4, 0:128],
                scalar1=-1.0, scalar2=1.0, op0=ALU.mult, op1=ALU.add)
            for ch2, xps in ((0, xwx_lo), (1, xwx_hi)):
                nc.tensor.matmul(
                    xps, lhsT=wx5_all[:, st, :],
                    rhs=xn_sb[:, st, ch2 * 512:(ch2 + 1) * 512],
                    start=(st == 0), stop=(st == n_st - 1))

        # Group-wise emission aligned to xT column chunks: each chunk's full
        # chain (rebase -> transposes -> wx middle -> xWx matmuls) is emitted
        # before the next pre-chunk, so the in-order PE queue never parks
        # ready xWx matmuls behind a pre-chunk still waiting on its columns.
        pre_chunk(0)
        for c in range(n_ch):
            pre_chunk(c + 1)
            rebase(c)
            for t in range(4 * c, 4 * c + 4):
                wx_s1(t)
            wx_s2(4 * c, 4 * c + 4)
            if c > 0:
                for t in range(4 * (c - 1), 4 * c):
                    wx_s3(t)
        for t in range(4 * (n_ch - 1), n_st):
            wx_s3(t)

        nc.scalar.activation(xwx_sb[:, 0:512], xwx_lo, AF.Copy)
        nc.vector.tensor_copy(xwx_sb[:, 512:1024], xwx_hi)

        # ---------- pairwise AllReduce #1: xWx5T ----------
        cc_in = dram.tile([5, D], fp32, tag="cc_in", name="cc_in")
        cc_out = dram.tile([5, D], fp32, tag="cc_out", name="cc_out")
        nc.sync.dma_start(cc_in[:, :], xwx_sb)
        if sim_no_cc:
            nc.gpsimd.dma_start(cc_out[:, :], cc_in[:, :])
        else:
            nc.gpsimd.collective_compute(
                "AllReduce", ALU.add,
                replica_groups=[[0, 1], [2, 3], [4, 5], [6, 7]],
                ins=[cc_in.opt()], outs=[cc_out.opt()])
        xwx_full = pt([5, D], "xwx_full")
        nc.sync.dma_start(xwx_full, cc_out[:, :])

        for wsb, outT in ((Wk_sb, kbT), (Wv_sb, vbT)):
            ps_kv = ps_a.tile([5, 512], fp32, tag="a", name="ps_kv")
            for ct in range(n_dt):
                nc.tensor.matmul(ps_kv[0:4, :], lhsT=abuf[:, ct, 96:100],
                                 rhs=wsb[:, ct, :],
                                 start=(ct == 0), stop=(ct == n_dt - 1))
            nc.scalar.activation(outT[0:4, :], ps_kv[0:4, :], AF.Copy, scale=0.5)

        # vb6[j, h, 0:5] = vbT[:, h*64+j]^T, vb6[:, :, 5] = 1 (rowsum column)
        vb6 = pt([64, H_LOC, 6], "vb6")
        nc.vector.memset(vb6, 1.0)
        for blk in range(n_dt_h):
            vb_ps = ps_tiny.tile([128, 8], fp32, tag="tiny", name="vb_ps")
            nc.tensor.transpose(
                vb_ps[:, 0:5], vbT[:, blk * 128:(blk + 1) * 128], ident[0:5, 0:5])
            nc.vector.tensor_copy(vb6[:, 2 * blk, 0:5], vb_ps[0:64, 0:5])
            nc.vector.tensor_copy(vb6[:, 2 * blk + 1, 0:5], vb_ps[64:128, 0:5])



        # transpose to [d-part, 5] tiles, folding in the attention scale
        xwx5 = pt([128, n_dt, 5], "xwx5", bf16)
        for ct in range(n_dt):
            xwt_ps = ps_tiny.tile([128, 8], fp32, tag="tiny", name="xwt_ps")
            nc.tensor.transpose(
                xwt_ps[:, 0:5], xwx_full[0:5, ct * 128:(ct + 1) * 128],
                ident[0:5, 0:5])
            nc.scalar.activation(xwx5[:, ct, :], xwt_ps[:, 0:5], AF.Copy,
                                 scale=float(SCALE))

        # ---------- phase B (own head half: 512 channel columns) ----------
        # kbT/vbT only depend on featBD + Wk/Wv: issue before qaT.
        qaT = pt([5, DH_LOC], "qaT", f32r)
        ps_q = ps_a.tile([5, 512], fp32, tag="a", name="ps_q")
        for ct in range(n_dt):
            nc.tensor.matmul(ps_q, lhsT=xwx5[:, ct, :], rhs=Wq_sb[:, ct, :],
                             start=(ct == 0), stop=(ct == n_dt - 1))
        nc.scalar.activation(qaT[:, :], ps_q, AF.Copy)

        # ---------- scores^T + exp + fused rowsum + normalize (8 local heads) ---
        sc_ps = ps_a.tile([64, H_LOC, DH], fp32, tag="a", name="sc_ps")
        for h in range(H_LOC):
            hs = slice(h * DH, (h + 1) * DH)
            nc.tensor.matmul(sc_ps[:, h, :], lhsT=kbT[:, hs], rhs=qaT[:, hs],
                             start=True, stop=True)
        ET = pt([64, H_LOC, DH], "ET")
        nc.scalar.activation(ET, sc_ps, AF.Exp)


        ps_u = ps_tiny.tile([64, H_LOC, 6], fp32, tag="tiny", name="ps_u")
        for h in range(H_LOC):
            nc.tensor.matmul(ps_u[:, h, :], lhsT=ET[:, h, :], rhs=vb6[:, h, :],
                             start=True, stop=True)
        rc = small.tile([64, H_LOC], fp32, name="rc")
        nc.vector.reciprocal(rc, ps_u[:, :, 5:6])
        Astack = pt([128, n_dt_h, 5], "Astack", bf16)
        for h in range(H_LOC):
            po = (h % 2) * 64
            nc.vector.tensor_scalar(
                out=Astack[po:po + 64, h // 2, :], in0=ps_u[:, h, 0:5],
                scalar1=rc[:, h:h + 1], scalar2=None, op0=ALU.mult)

        # ---------- partial MT over own d-half -> AllReduce #2 -> Mc6 ----------
        mt_sb = pt([5, D], "mt_sb", f32r)
        for ch in range(2):
            sl = slice(ch * 512, (ch + 1) * 512)
            ps_m = ps_a.tile([5, 512], fp32, tag="a", name="ps_m")
            for ct in range(n_dt_h):
                nc.tensor.matmul(ps_m, lhsT=Astack[:, ct, :],
                                 rhs=Wo_sb[:, ct, sl],
                                 start=(ct == 0), stop=(ct == n_dt_h - 1))
            # fold bo/2 in pre-AllReduce (both pair members add half) so the
            # readback can DMA straight into Mc6 rows 0-4 with no post-AR add
            nc.vector.scalar_tensor_tensor(
                out=mt_sb[:, sl], in0=ps_m, scalar=1.0, in1=bo5[:, sl],
                op0=ALU.mult, op1=ALU.add)

        cc2_in = dram.tile([5, D], f32r, tag="cc2_in", name="cc2_in")
        cc2_out = dram.tile([5, D], f32r, tag="cc2_out", name="cc2_out")
        nc.sync.dma_start(cc2_in[:, :], mt_sb)
        if sim_no_cc:
            nc.gpsimd.dma_start(cc2_out[:, :], cc2_in[:, :])
        else:
            nc.gpsimd.collective_compute(
                "AllReduce", ALU.add,
                replica_groups=[[0, 1], [2, 3], [4, 5], [6, 7]],
                ins=[cc2_in.opt()], outs=[cc2_out.opt()])
        # accumulate the reduced MT directly onto the bo-seeded Mc6 rows,
        # lo half first so the y matmuls on columns 0-511 unblock earlier
        # lo half first so the y matmuls on columns 0-511 unblock earlier
        nc.sync.dma_start(Mc6[0:5, 0:512], cc2_out[:, 0:512])
        nc.sync.dma_start(Mc6[0:5, 512:1024], cc2_out[:, 512:1024])

        # ---------- phase C: y = wx6T^T @ Mc6; ramped groups so the first
        # store issues after ~2 tiles while later groups amortize DMA count --
        groups = [(0, 1), (1, 1), (2, 2), (4, 3), (7, 3), (10, 3), (13, 3)]
        for g0, gn in groups:
            y_sb = ypool.tile([128, gn, D], fp32, name="y_sb")
            for j in range(gn):
                st = g0 + j
                y_lo = ps_a.tile([128, 512], fp32, tag="a", name="y_lo")
                y_hi = ps_a.tile([128, 512], fp32, tag="a", name="y_hi")
                wsl = wx6T[:, st * 128:(st + 1) * 128]
                nc.tensor.matmul(y_lo, lhsT=wsl, rhs=Mc6[:, 0:512],
                                 start=True, stop=True)
                nc.tensor.matmul(y_hi, lhsT=wsl, rhs=Mc6[:, 512:1024],
                                 start=True, stop=True)
                # split the PSUM->SBUF copies across scalar + vector engines
                nc.scalar.activation(y_sb[:, j, 0:512], y_lo, AF.Copy)
                nc.vector.tensor_copy(y_sb[:, j, 512:1024], y_hi)
            nc.sync.dma_start(y_p[:, g0:g0 + gn, :], y_sb)

    return nc


def _prep_host(inputs, s_sh):
    x = np.ascontiguousarray(np.asarray(inputs["x"], dtype=np.float32))
    Wq = np.asarray(inputs["Wq"], np.float32)
    Wk = np.asarray(inputs["Wk"], np.float32)
    Wv = np.asarray(inputs["Wv"], np.float32)
    Wo = np.asarray(inputs["Wo"], np.float32)
    bq = np.asarray(inputs["bq"], np.float32)
    bk = np.asarray(inputs["bk"], np.float32)
    bv = np.asarray(inputs["bv"], np.float32)
    bo = np.asarray(inputs["bo"], np.float32)
    Woff1 = np.asarray(inputs["Woff1"], np.float32)
    boff1 = np.asarray(inputs["boff1"], np.float32)
    Woff2 = np.asarray(inputs["Woff2"], np.float32)
    bt = np.asarray(inputs["bias_table"], np.float32)[0, 0]

    assert np.all(bq == 0.0), "nonzero bq not supported by this kernel"

    n_st = s_sh // 128
    n_dt = D // 128
    n_dt_h = n_dt // 2

    w_eff = np.einsum("o,ock->ck", Woff2, Woff1)            # [DG, K]
    # AoffP[d, 32k+g] = a_{g,k}[d]; k-blocks padded to 32 so the DVE shift-adds
    # land on partition starts 0/32/64
    AoffP = np.zeros((D, K, 32), np.float32)
    for g in range(G):
        blk = Wq[g * DG:(g + 1) * DG, :]                    # [DG, D]
        for k in range(K):
            AoffP[:, k, g] = w_eff[:, k] @ blk
    AoffP = AoffP.reshape(D, 96)
    offconst = float(Woff2 @ boff1)

    WqT = np.ascontiguousarray(Wq.T)
    WkT = np.ascontiguousarray(Wk.T)
    WvT = np.ascontiguousarray(Wv.T)
    WoT = np.ascontiguousarray(Wo.T)

    def pack_dmaj(a, cols):      # [D, cols] -> [128, n_dt, cols]
        return np.ascontiguousarray(
            a.reshape(-1, 128, cols).transpose(1, 0, 2))

    base_full = np.arange(S, dtype=np.float32) / (S - 1) - 0.5

    in_maps = []
    for c in range(NCORES):
        b = c // 2
        hf = c % 2
        s0 = hf * s_sh
        hsl = slice(hf * DH_LOC, (hf + 1) * DH_LOC)
        xb = x[b]
        xT = np.zeros((D, s_sh + 2), np.float32)
        lo = max(s0 - 1, 0)
        hi = min(s0 + s_sh + 1, S)
        xT[:, lo - (s0 - 1): hi - (s0 - 1)] = xb[lo:hi].T

        # abuf: aoff columns 0:96, featBD (group-scattered feat rows) 96:100
        feat2 = xb[2047] + xb[2048]                          # [D] (0.5 on-chip)
        featBD = np.zeros((D, G), np.float32)
        for g in range(G):
            featBD[g * DG:(g + 1) * DG, g] = feat2[g * DG:(g + 1) * DG]
        ab = np.concatenate([AoffP, featBD], axis=1)         # [D, 100]

        sm1r = np.concatenate([bt[s0:s0 + s_sh], bk[hsl], Wo.sum(axis=1),
                               np.ones(s_sh, np.float32)])[None, :]
        sm1f = np.concatenate([bv[hsl], 0.5 * bo])[None, :]

        m = {
            "xT_p": pack_dmaj(xT, s_sh + 2).astype(ml_dtypes.bfloat16),
            "xn_p": np.ascontiguousarray(
                xb[s0:s0 + s_sh].reshape(n_st, 128, D).transpose(1, 0, 2)
            ).astype(ml_dtypes.bfloat16),
            "ab_p": pack_dmaj(ab, 100).astype(ml_dtypes.bfloat16),
            "Wq_p": pack_dmaj(WqT[:, hsl], DH_LOC).astype(ml_dtypes.bfloat16),
            "Wk_p": pack_dmaj(WkT[:, hsl], DH_LOC).astype(ml_dtypes.bfloat16),
            "Wv_p": pack_dmaj(WvT[:, hsl], DH_LOC).astype(ml_dtypes.bfloat16),
            "Wo_p": np.ascontiguousarray(
                WoT[hsl, :].reshape(n_dt_h, 128, D).transpose(1, 0, 2)
            ).astype(ml_dtypes.bfloat16),
            "sm1r": np.ascontiguousarray(sm1r),
            "sm1f": np.ascontiguousarray(sm1f),
            "sm2": np.ascontiguousarray(np.repeat(
                base_full[s0:s0 + s_sh].reshape(n_st, 128).T[:, :, None],
                4, axis=2)),
        }
        in_maps.append(m)
    return in_maps, offconst


def _get_nc(s_sh, offconst):
    key = (s_sh, offconst)
    if key not in _CACHE:
        nc = _build_bass(s_sh, offconst)
        nc.finalize()   # Bacc: runs wait-splitting + register allocation
        _CACHE[key] = nc
    return _CACHE[key]


S_SH = S // 2


def kernel(**inputs) -> np.ndarray:
    from concourse.bass_utils import run_bass_kernel_spmd

    in_maps, offconst = _prep_host(inputs, S_SH)
    nc = _get_nc(S_SH, offconst)
    res = run_bass_kernel_spmd(nc, in_maps, core_ids=list(range(NCORES)))
    y = np.zeros((B, S, D), np.float32)
    n_st = S_SH // 128
    for c in range(NCORES):
        b = c // 2
        hf = c % 2
        yc = res.results[c]["y"]          # [128, n_st, D]
        y[b, hf * S_SH:(hf + 1) * S_SH] = (
            yc.transpose(1, 0, 2).reshape(S_SH, D))
    return y


if __name__ == "__main__":
    import reference
    inputs = {k: np.asarray(v) for k, v in reference.setup_inputs().items()}
    got = kernel(**inputs)
    import jax.numpy as jnp
    exp = np.asarray(reference.reference(**{k: jnp.asarray(v) for k, v in inputs.items()}))
    rel = np.linalg.norm(got - exp) / np.linalg.norm(exp)
    print("Relative error:", rel)


# revision 55
# speedup vs baseline: 1.0134x; 1.0134x over previous
"""Trainium2 Bass kernel for nn_DeformAtten1D (B=4, S=4096, D=1024, H=16, G=4, K=3).

Math: the reference's grid-sample degenerates (iy = (S-1)/2 fixed, width dim = 1), so
x_sampled = feat_c (outer) wx  is rank-1 per (batch, group).  Propagating that structure
collapses every large GEMM:

  offset[g,s] = sum_k a_{g,k} . x[s+k-1,:]      (a_{g,k} = Wq_g^T @ w_eff_k, weight-only)
  wx[g,s]     = 1 - |tanh(offset)*K/(S-1) + s/(S-1) - 0.5|      (clip provably inactive)
  xWx5T       = [wx;1] @ x                      [5, D]   (the only s-reduction over x)
  qaT         = scale * xWx5T @ Wq^T            [5, D]
  kbT/vbT     = [0.5*featBD^T @ W^T ; bias]     [5, D]
  scT_h       = kbT_h^T @ qaT_h   (scores transposed; exp safe without max-sub
                since |scores| < 5 for this operator scale)
  U_h         = exp(scT_h) @ [vb_h | 1]  ->  Astack_h = U[:,0:5] / U[:,5]
  MT          = Astack^T @ Wo^T + bo/2 (pre-AllReduce: both pair members
                add half, so the reduced rows land ready-to-use)
  Mc6         = [AllReduce(MT) ; Wo@1]   (readback DMAs straight into Mc6)
  y[s,:]      = [wx[:,s]; 1; bt[s]]^T @ Mc6     (bias_table term: attn row-sums == 1)

Sharding: core c -> (batch c//2, sequence half c%2), S_SH=2048; attention heads are
additionally split across the pair (even core: heads 0-7, odd: 8-15) so each core
reads only half of each projection weight.  Cross-core data: two pairwise AllReduces
of [5,1024] partials (xWx5T after phase A, MT after phase B).

Perf notes (123.6us -> 94.1us on the TimelineSim cost model):
 - DMA count collapsed ~110 -> ~30: inputs are host-packed partition-major
   ([128, tiles, cols]) and loaded as a handful of large transfers (each DMA
   costs ~625ns serialized HWDGE issue + ~900ns completion latency in the
   model, and all transfers serialize on one DMA_ENGINES resource).
 - x (both layouts) and all four projection-weight halves load as bf16,
   halving input HBM bytes (end-to-end error validated ~4e-3 vs 2e-2 budget).
 - xT/xn stream in interleaved 512-column chunks so phase A starts after ~3us
   and the xWx accumulation chases the arriving data.
 - Phase A is emitted chunk-pipelined and stage-split (in-order engine queues:
   never park a ready matmul behind a stalled one); the wx middle is batched
   [128, w, 4] to amortize the ~220-cycle per-op engine access overhead.
 - Attention avoids per-head softmax chains: scores are built transposed
   (lhsT=kb, rhs=qa), exp needs no max-subtraction (|scores| < 5), the row-sum
   rides as a ones-column through the value matmul, and the normalization is a
   per-partition reciprocal+scale.  kb/vb/vb6 are emitted before the AR1
   readback to fill the collective's latency window.
 - Phase C writes y through ramped store groups (1,1,2,3,3,3,3 tiles) so the
   first store issues early while later groups amortize DMA issue cost.
Hardware runtime pitfalls baked in: PE cannot read lhsT from partition base
32/64 (runtime crash) -- the k-shift rebase uses DVE cross-partition-base
copies instead; GPSIMD cannot touch PSUM; DVE ops may read at most one PSUM
operand; f32r memsets and strided memsets fail the ISA check.
"""

import numpy as np
import ml_dtypes

B, S, D, H, G, K = 4, 4096, 1024, 16, 4, 3
DG, DH = D // G, D // H
NCORES = 8
SCALE = D ** (-0.5)
H_LOC = H // 2          # heads per core (pair-split)
DH_LOC = H_LOC * DH     # 512 channel columns per core

_CACHE = {}


def _build_bass(s_sh: int, offconst: float, sim_no_cc: bool = False):
    from contextlib import ExitStack
    import concourse.bass as bass
    import concourse.mybir as mybir
    import concourse.tile as tile
    from concourse import bacc
    from concourse.masks import make_identity

    fp32 = mybir.dt.float32
    f32r = mybir.dt.float32r
    bf16 = mybir.dt.bfloat16
    AF = mybir.ActivationFunctionType
    ALU = mybir.AluOpType

    n_st = s_sh // 128          # s-tiles (16)
    n_ch = s_sh // 512          # 512-wide chunks (4)
    n_dt = D // 128             # d-tiles (8)
    n_dt_h = n_dt // 2          # d-tiles of this core's head half (4)
    W = s_sh + 2                # halo width (2050)

    nc = bacc.Bacc(None, num_devices=NCORES)

    # --- host-packed DRAM inputs (partition-major: [128, tiles, cols]) ---
    xT_p = nc.declare_dram_parameter("xT_p", [128, n_dt, W], bf16, isOutput=False)
    xn_p = nc.declare_dram_parameter("xn_p", [128, n_st, D], bf16, isOutput=False)
    ab_p = nc.declare_dram_parameter("ab_p", [128, n_dt, 100], bf16, isOutput=False)
    Wq_p = nc.declare_dram_parameter("Wq_p", [128, n_dt, DH_LOC], bf16, isOutput=False)
    Wk_p = nc.declare_dram_parameter("Wk_p", [128, n_dt, DH_LOC], bf16, isOutput=False)
    Wv_p = nc.declare_dram_parameter("Wv_p", [128, n_dt, DH_LOC], bf16, isOutput=False)
    Wo_p = nc.declare_dram_parameter("Wo_p", [128, n_dt_h, D], bf16, isOutput=False)
    # sm1r: [bt(2048) | bk(512) | w1(1024) | ones(s_sh)] (f32r destinations)
    sm1r = nc.declare_dram_parameter("sm1r", [1, 2 * s_sh + 1536], f32r,
                                     isOutput=False)
    # sm1f: [bv(512) | bo(1024)] on partition 0 (fp32 destinations)
    sm1f = nc.declare_dram_parameter("sm1f", [1, 1536], fp32, isOutput=False)
    sm2 = nc.declare_dram_parameter("sm2", [128, n_st, 4], fp32, isOutput=False)
    y_p = nc.declare_dram_parameter("y", [128, n_st, D], fp32, isOutput=True)

    with tile.TileContext(nc) as tc, ExitStack() as ctx:
        P = ctx.enter_context(tc.tile_pool(name="persist", bufs=1))
        small = ctx.enter_context(tc.tile_pool(name="small", bufs=24))
        ypool = ctx.enter_context(tc.tile_pool(name="ypool", bufs=2))
        ps_a = ctx.enter_context(tc.tile_pool(name="ps_a", bufs=3, space="PSUM"))
        ps_fix = ctx.enter_context(tc.tile_pool(name="ps_fix", bufs=1, space="PSUM"))
        ps_pre = ctx.enter_context(tc.tile_pool(name="ps_pre", bufs=2, space="PSUM"))
        ps_tiny = ctx.enter_context(tc.tile_pool(name="ps_tiny", bufs=2, space="PSUM"))
        dram = ctx.enter_context(tc.tile_pool(name="dram", bufs=1, space="DRAM"))

        def pt(shape, tag, dtype=fp32):
            return P.tile(shape, dtype, tag=tag, name=tag)

        # ---------- constants / input DMAs (issue order == DMA queue order) ----
        ident = pt([128, 128], "ident")
        make_identity(nc, ident)

        # abuf first (preT lhsT), then xT in 4 chunks so the preT accumulation
        # starts after ~3us instead of waiting for one monolithic 12us DMA.
        abuf = pt([128, n_dt, 100], "abuf", bf16)   # [:, :, 0:96]=aoff, 96:100=featBD
        nc.sync.dma_start(abuf, ab_p[:, :, :])

        base_sb = pt([128, n_st, 4], "base")
        nc.sync.dma_start(base_sb, sm2[:, :, :])

        wx6T = pt([6, s_sh], "wx6T", f32r)          # rows 0-3 wx, 4 ones, 5 bias_table
        nc.sync.dma_start(wx6T[5:6, :], sm1r[:, 0:s_sh])
        nc.sync.dma_start(wx6T[4:5, :],
                          sm1r[:, s_sh + 1536:2 * s_sh + 1536])

        # xT and xn interleaved in COLUMN chunks: pre-chunk c (and the wx
        # tiles + xWx accumulation behind it) unblocks after ~3us*(c+1)
        # instead of waiting for the full 12us xT transfer.
        xT_sb = pt([128, n_dt, W], "xT", bf16)
        xn_sb = pt([128, n_st, D], "xn", bf16)
        kbT = pt([5, DH_LOC], "kbT", f32r)
        vbT = pt([5, DH_LOC], "vbT")
        Mc6 = pt([6, D], "Mc6", f32r)
        bo5 = pt([5, D], "bo5")
        nc.vector.memset(bo5, 0.0)

        xt_cuts = (0, 512, 1024, 1536, W)
        for q in range(4):
            nc.sync.dma_start(xT_sb[:, :, xt_cuts[q]:xt_cuts[q + 1]],
                              xT_p[:, :, xt_cuts[q]:xt_cuts[q + 1]])
            nc.sync.dma_start(xn_sb[:, 4 * q:4 * q + 4, :],
                              xn_p[:, 4 * q:4 * q + 4, :])
            if q == 0:
                # bias rows slot in behind the first x chunks; needed late
                nc.sync.dma_start(kbT[4:5, :], sm1r[:, s_sh:s_sh + 512])
                nc.sync.dma_start(vbT[4:5, :], sm1f[:, 0:512])
                nc.sync.dma_start(Mc6[5:6, :], sm1r[:, s_sh + 512:s_sh + 1536])
                nc.sync.dma_start(bo5[4:5, :], sm1f[:, 512:1536])

        Wk_sb = pt([128, n_dt, DH_LOC], "Wk", bf16)
        nc.sync.dma_start(Wk_sb, Wk_p[:, :, :])
        Wv_sb = pt([128, n_dt, DH_LOC], "Wv", bf16)
        nc.sync.dma_start(Wv_sb, Wv_p[:, :, :])
        Wq_sb = pt([128, n_dt, DH_LOC], "Wq", bf16)
        nc.sync.dma_start(Wq_sb, Wq_p[:, :, :])
        Wo_sb = pt([128, n_dt_h, D], "Wo", bf16)
        nc.sync.dma_start(Wo_sb, Wo_p[:, :, :])

        # ---------- phase A: preT[32k+g, w] = sum_d a_{g,k}[d] * xT[d, w] -------
        # The k=0/1/2 blocks live at partition bases 0/32/64; the +1/+2 column
        # shifts of the depthwise conv are folded into per-s-tile PE transposes
        # (three [4,128+shift] -> [128,4] transposes into one PSUM tile), so no
        # mid-phase DMAs enter the (FIFO) DMA queue behind the weight loads.
        preT = pt([96, W], "preT")
        preS1 = pt([4, s_sh], "preS1")
        preS2 = pt([4, s_sh], "preS2")
        wx5_all = pt([128, n_st, 5], "wx5", bf16)
        wxa_all = pt([128, n_st, 4], "wxa")
        # contiguous full-tile memset (strided memset fails the ISA check);
        # columns 0-3 are overwritten per tile, column 4 stays == 1
        nc.vector.memset(wx5_all, 1.0)
        tt_all = ps_fix.tile([128, n_st, 12], fp32, tag="tt", name="tt_all")
        xwx_lo = ps_a.tile([5, 512], fp32, tag="a", name="xwx_lo")
        xwx_hi = ps_a.tile([5, 512], fp32, tag="a", name="xwx_hi")
        xwx_sb = pt([5, D], "xwx_sb")

        def pre_chunk(c):
            w0 = c * 512
            nw = 512 if c < n_ch else 2        # tail: last 2 halo columns
            pre_ps = ps_pre.tile([96, 512], fp32, tag="pre", name="pre_ps")
            for ct in range(n_dt):
                nc.tensor.matmul(
                    pre_ps[:, 0:nw], lhsT=abuf[:, ct, 0:96],
                    rhs=xT_sb[:, ct, w0: w0 + nw],
                    start=(ct == 0), stop=(ct == n_dt - 1))
            nc.scalar.activation(preT[:, w0:w0 + nw], pre_ps[:, 0:nw], AF.Copy)

        def rebase(c):
            # DVE cross-partition-base copies fold the +1/+2 column shifts of
            # the k=1/k=2 blocks down to partitions 0-3 (PE cannot read lhsT
            # from partition base 32/64 -- that crashes the runtime)
            w0 = c * 512
            nc.vector.tensor_copy(preS1[:, w0:w0 + 512],
                                  preT[32:36, w0 + 1:w0 + 513])
            nc.vector.tensor_copy(preS2[:, w0:w0 + 512],
                                  preT[64:68, w0 + 2:w0 + 514])

        def wx_s1(st):
            # stage 1 (PE): three k-block transposes into tt_all[st]
            c0 = st * 128
            for k, srcb in enumerate((preT, preS1, preS2)):
                nc.tensor.transpose(
                    tt_all[:, st, 4 * k:4 * k + 4],
                    srcb[0:4, c0:c0 + 128], ident[0:4, 0:4])

        def wx_s2(t0, t1):
            # stage 2, batched over tiles [t0, t1): shift-add, tanh,
            # |K/(S-1)*x + base|, 1-x.  Batching amortizes the ~220-cycle
            # per-op SBUF access overhead that dominated [128,4]-shaped ops.
            w = t1 - t0
            # hw limit: at most one PSUM operand per instruction -- stage the
            # k=0 block through SBUF, then chain single-PSUM adds
            c0 = small.tile([128, w, 4], fp32, name="c0")
            nc.vector.tensor_copy(c0, tt_all[:, t0:t1, 0:4])
            s01 = small.tile([128, w, 4], fp32, name="s01")
            nc.vector.tensor_add(out=s01, in0=c0, in1=tt_all[:, t0:t1, 4:8])
            s012 = small.tile([128, w, 4], fp32, name="s012")
            nc.vector.tensor_add(out=s012, in0=s01, in1=tt_all[:, t0:t1, 8:12])
            tgt = small.tile([128, w, 4], fp32, name="tgt")
            nc.scalar.activation(tgt, s012, AF.Tanh, bias=float(offconst))
            u = small.tile([128, w, 4], fp32, name="u")
            nc.vector.scalar_tensor_tensor(
                out=u, in0=tgt, scalar=float(K / (S - 1)),
                in1=base_sb[:, t0:t1, :], op0=ALU.mult, op1=ALU.add)
            a = wxa_all[:, t0:t1, :]
            nc.scalar.activation(a, u, AF.Abs)
            nc.scalar.activation(wx5_all[:, t0:t1, 0:4], a, AF.Copy,
                                 scale=-1.0, bias=1.0)

        def wx_s3(st):
            # stage 3 (PE+DVE): transpose |.| back, 1-x on the way into wx6T
            # rows 0-3 (row 4 is DMA-loaded ones); xWx accumulation
            wT_ps = ps_tiny.tile([4, 128], fp32, tag="tiny", name="wT_ps")
            nc.tensor.transpose(wT_ps[0:4, 0:128], wxa_all[:, st, :], ident)
            nc.vector.tensor_scalar(
                out=wx6T[0:4, st * 128:(st + 1) * 128], in0=wT_ps[0:4, 0:128],
                scalar1=-1.0, scalar2=1.0, op0=ALU.mult, op1=ALU.add)
            for ch2, xps in ((0, xwx_lo), (1, xwx_hi)):
                nc.tensor.matmul(
                    xps, lhsT=wx5_all[:, st, :],
                    rhs=xn_sb[:, st, ch2 * 512:(ch2 + 1) * 512],
                    start=(st == 0), stop=(st == n_st - 1))

        # Group-wise emission aligned to xT column chunks: each chunk's full
        # chain (rebase -> transposes -> wx middle -> xWx matmuls) is emitted
        # before the next pre-chunk, so the in-order PE queue never parks
        # ready xWx matmuls behind a pre-chunk still waiting on its columns.
        pre_chunk(0)
        for c in range(n_ch):
            pre_chunk(c + 1)
            rebase(c)
            for t in range(4 * c, 4 * c + 4):
                wx_s1(t)
            wx_s2(4 * c, 4 * c + 4)
            if c > 0:
                for t in range(4 * (c - 1), 4 * c):
                    wx_s3(t)
        for t in range(4 * (n_ch - 1), n_st):
            wx_s3(t)

        nc.scalar.activation(xwx_sb[:, 0:512], xwx_lo, AF.Copy)
        nc.vector.tensor_copy(xwx_sb[:, 512:1024], xwx_hi)

        # ---------- pairwise AllReduce #1: xWx5T ----------
        cc_in = dram.tile([5, D], fp32, tag="cc_in", name="cc_in")
        cc_out = dram.tile([5, D], fp32, tag="cc_out", name="cc_out")
        nc.sync.dma_start(cc_in[:, :], xwx_sb)
        if sim_no_cc:
            nc.gpsimd.dma_start(cc_out[:, :], cc_in[:, :])
        else:
            nc.gpsimd.collective_compute(
                "AllReduce", ALU.add,
                replica_groups=[[0, 1], [2, 3], [4, 5], [6, 7]],
                ins=[cc_in.opt()], outs=[cc_out.opt()])
        xwx_full = pt([5, D], "xwx_full")
        nc.sync.dma_start(xwx_full, cc_out[:, :])

        for wsb, outT in ((Wk_sb, kbT), (Wv_sb, vbT)):
            ps_kv = ps_a.tile([5, 512], fp32, tag="a", name="ps_kv")
            for ct in range(n_dt):
                nc.tensor.matmul(ps_kv[0:4, :], lhsT=abuf[:, ct, 96:100],
                                 rhs=wsb[:, ct, :],
                                 start=(ct == 0), stop=(ct == n_dt - 1))
            nc.scalar.activation(outT[0:4, :], ps_kv[0:4, :], AF.Copy, scale=0.5)

        # vb6[j, h, 0:5] = vbT[:, h*64+j]^T, vb6[:, :, 5] = 1 (rowsum column)
        vb6 = pt([64, H_LOC, 6], "vb6")
        nc.vector.memset(vb6, 1.0)
        for blk in range(n_dt_h):
            vb_ps = ps_tiny.tile([128, 8], fp32, tag="tiny", name="vb_ps")
            nc.tensor.transpose(
                vb_ps[:, 0:5], vbT[:, blk * 128:(blk + 1) * 128], ident[0:5, 0:5])
            nc.vector.tensor_copy(vb6[:, 2 * blk, 0:5], vb_ps[0:64, 0:5])
            nc.vector.tensor_copy(vb6[:, 2 * blk + 1, 0:5], vb_ps[64:128, 0:5])



        # transpose to [d-part, 5] tiles, folding in the attention scale
        xwx5 = pt([128, n_dt, 5], "xwx5", bf16)
        for ct in range(n_dt):
            xwt_ps = ps_tiny.tile([128, 8], fp32, tag="tiny", name="xwt_ps")
            nc.tensor.transpose(
                xwt_ps[:, 0:5], xwx_full[0:5, ct * 128:(ct + 1) * 128],
                ident[0:5, 0:5])
            nc.scalar.activation(xwx5[:, ct, :], xwt_ps[:, 0:5], AF.Copy,
                                 scale=float(SCALE))

        # ---------- phase B (own head half: 512 channel columns) ----------
        # kbT/vbT only depend on featBD + Wk/Wv: issue before qaT.
        qaT = pt([5, DH_LOC], "qaT", f32r)
        ps_q = ps_a.tile([5, 512], fp32, tag="a", name="ps_q")
        for ct in range(n_dt):
            nc.tensor.matmul(ps_q, lhsT=xwx5[:, ct, :], rhs=Wq_sb[:, ct, :],
                             start=(ct == 0), stop=(ct == n_dt - 1))
        nc.scalar.activation(qaT[:, :], ps_q, AF.Copy)

        # ---------- scores^T + exp + fused rowsum + normalize (8 local heads) ---
        sc_ps = ps_a.tile([64, H_LOC, DH], fp32, tag="a", name="sc_ps")
        for h in range(H_LOC):
            hs = slice(h * DH, (h + 1) * DH)
            nc.tensor.matmul(sc_ps[:, h, :], lhsT=kbT[:, hs], rhs=qaT[:, hs],
                             start=True, stop=True)
        ET = pt([64, H_LOC, DH], "ET")
        nc.scalar.activation(ET, sc_ps, AF.Exp)


        ps_u = ps_tiny.tile([64, H_LOC, 6], fp32, tag="tiny", name="ps_u")
        for h in range(H_LOC):
            nc.tensor.matmul(ps_u[:, h, :], lhsT=ET[:, h, :], rhs=vb6[:, h, :],
                             start=True, stop=True)
        rc = small.tile([64, H_LOC], fp32, name="rc")
        nc.vector.reciprocal(rc, ps_u[:, :, 5:6])
        Astack = pt([128, n_dt_h, 5], "Astack", bf16)
        for h in range(H_LOC):
            po = (h % 2) * 64
            nc.vector.tensor_scalar(
                out=Astack[po:po + 64, h // 2, :], in0=ps_u[:, h, 0:5],
                scalar1=rc[:, h:h + 1], scalar2=None, op0=ALU.mult)

        # ---------- partial MT over own d-half -> AllReduce #2 -> Mc6 ----------
        mt_sb = pt([5, D], "mt_sb", f32r)
        for ch in range(2):
            sl = slice(ch * 512, (ch + 1) * 512)
            ps_m = ps_a.tile([5, 512], fp32, tag="a", name="ps_m")
            for ct in range(n_dt_h):
                nc.tensor.matmul(ps_m, lhsT=Astack[:, ct, :],
                                 rhs=Wo_sb[:, ct, sl],
                                 start=(ct == 0), stop=(ct == n_dt_h - 1))
            # fold bo/2 in pre-AllReduce (both pair members add half) so the
            # readback can DMA straight into Mc6 rows 0-4 with no post-AR add
            nc.vector.scalar_tensor_tensor(
                out=mt_sb[:, sl], in0=ps_m, scalar=1.0, in1=bo5[:, sl],
                op0=ALU.mult, op1=ALU.add)

        cc2_in = dram.tile([5, D], f32r, tag="cc2_in", name="cc2_in")
        cc2_out = dram.tile([5, D], f32r, tag="cc2_out", name="cc2_out")
        nc.sync.dma_start(cc2_in[:, :], mt_sb)
        if sim_no_cc:
            nc.gpsimd.dma_start(cc2_out[:, :], cc2_in[:, :])
        else:
            nc.gpsimd.collective_compute(
                "AllReduce", ALU.add,
                replica_groups=[[0, 1], [2, 3], [4, 5], [6, 7]],
                ins=[cc2_in.opt()], outs=[cc2_out.opt()])
        # accumulate the reduced MT directly onto the bo-seeded Mc6 rows,
        # lo half first so the y matmuls on columns 0-511 unblock earlier
        # lo half first so the y matmuls on columns 0-511 unblock earlier
        nc.sync.dma_start(Mc6[0:5, 0:512], cc2_out[:, 0:512])
        nc.sync.dma_start(Mc6[0:5, 512:1024], cc2_out[:, 512:1024])

        # ---------- phase C: y = wx6T^T @ Mc6; ramped groups so the first
        # store issues after ~2 tiles while later groups amortize DMA count --
        groups = [(0, 1), (1, 2), (3, 2), (5, 3), (8, 4), (12, 4)]
        for g0, gn in groups:
            y_sb = ypool.tile([128, gn, D], fp32, name="y_sb")
            for j in range(gn):
                st = g0 + j
                y_lo = ps_a.tile([128, 512], fp32, tag="a", name="y_lo")
                y_hi = ps_a.tile([128, 512], fp32, tag="a", name="y_hi")
                wsl = wx6T[:, st * 128:(st + 1) * 128]
                nc.tensor.matmul(y_lo, lhsT=wsl, rhs=Mc6[:, 0:512],
                                 start=True, stop=True)
                nc.tensor.matmul(y_hi, lhsT=wsl, rhs=Mc6[:, 512:1024],
                                 start=True, stop=True)
                # split the PSUM->SBUF copies across scalar + vector engines
                nc.scalar.activation(y_sb[:, j, 0:512], y_lo, AF.Copy)
                nc.vector.tensor_copy(y_sb[:, j, 512:1024], y_hi)
            nc.sync.dma_start(y_p[:, g0:g0 + gn, :], y_sb)

    return nc


def _prep_host(inputs, s_sh):
    x = np.ascontiguousarray(np.asarray(inputs["x"], dtype=np.float32))
    Wq = np.asarray(inputs["Wq"], np.float32)
    Wk = np.asarray(inputs["Wk"], np.float32)
    Wv = np.asarray(inputs["Wv"], np.float32)
    Wo = np.asarray(inputs["Wo"], np.float32)
    bq = np.asarray(inputs["bq"], np.float32)
    bk = np.asarray(inputs["bk"], np.float32)
    bv = np.asarray(inputs["bv"], np.float32)
    bo = np.asarray(inputs["bo"], np.float32)
    Woff1 = np.asarray(inputs["Woff1"], np.float32)
    boff1 = np.asarray(inputs["boff1"], np.float32)
    Woff2 = np.asarray(inputs["Woff2"], np.float32)
    bt = np.asarray(inputs["bias_table"], np.float32)[0, 0]

    assert np.all(bq == 0.0), "nonzero bq not supported by this kernel"

    n_st = s_sh // 128
    n_dt = D // 128
    n_dt_h = n_dt // 2

    w_eff = np.einsum("o,ock->ck", Woff2, Woff1)            # [DG, K]
    # AoffP[d, 32k+g] = a_{g,k}[d]; k-blocks padded to 32 so the DVE shift-adds
    # land on partition starts 0/32/64
    AoffP = np.zeros((D, K, 32), np.float32)
    for g in range(G):
        blk = Wq[g * DG:(g + 1) * DG, :]                    # [DG, D]
        for k in range(K):
            AoffP[:, k, g] = w_eff[:, k] @ blk
    AoffP = AoffP.reshape(D, 96)
    offconst = float(Woff2 @ boff1)

    WqT = np.ascontiguousarray(Wq.T)
    WkT = np.ascontiguousarray(Wk.T)
    WvT = np.ascontiguousarray(Wv.T)
    WoT = np.ascontiguousarray(Wo.T)

    def pack_dmaj(a, cols):      # [D, cols] -> [128, n_dt, cols]
        return np.ascontiguousarray(
            a.reshape(-1, 128, cols).transpose(1, 0, 2))

    base_full = np.arange(S, dtype=np.float32) / (S - 1) - 0.5

    in_maps = []
    for c in range(NCORES):
        b = c // 2
        hf = c % 2
        s0 = hf * s_sh
        hsl = slice(hf * DH_LOC, (hf + 1) * DH_LOC)
        xb = x[b]
        xT = np.zeros((D, s_sh + 2), np.float32)
        lo = max(s0 - 1, 0)
        hi = min(s0 + s_sh + 1, S)
        xT[:, lo - (s0 - 1): hi - (s0 - 1)] = xb[lo:hi].T

        # abuf: aoff columns 0:96, featBD (group-scattered feat rows) 96:100
        feat2 = xb[2047] + xb[2048]                          # [D] (0.5 on-chip)
        featBD = np.zeros((D, G), np.float32)
        for g in range(G):
            featBD[g * DG:(g + 1) * DG, g] = feat2[g * DG:(g + 1) * DG]
        ab = np.concatenate([AoffP, featBD], axis=1)         # [D, 100]

        sm1r = np.concatenate([bt[s0:s0 + s_sh], bk[hsl], Wo.sum(axis=1),
                               np.ones(s_sh, np.float32)])[None, :]
        sm1f = np.concatenate([bv[hsl], 0.5 * bo])[None, :]

        m = {
            "xT_p": pack_dmaj(xT, s_sh + 2).astype(ml_dtypes.bfloat16),
            "xn_p": np.ascontiguousarray(
                xb[s0:s0 + s_sh].reshape(n_st, 128, D).transpose(1, 0, 2)
            ).astype(ml_dtypes.bfloat16),
            "ab_p": pack_dmaj(ab, 100).astype(ml_dtypes.bfloat16),
            "Wq_p": pack_dmaj(WqT[:, hsl], DH_LOC).astype(ml_dtypes.bfloat16),
            "Wk_p": pack_dmaj(WkT[:, hsl], DH_LOC).astype(ml_dtypes.bfloat16),
            "Wv_p": pack_dmaj(WvT[:, hsl], DH_LOC).astype(ml_dtypes.bfloat16),
            "Wo_p": np.ascontiguousarray(
                WoT[hsl, :].reshape(n_dt_h, 128, D).transpose(1, 0, 2)
            ).astype(ml_dtypes.bfloat16),
            "sm1r": np.ascontiguousarray(sm1r),
            "sm1f": np.ascontiguousarray(sm1f),
            "sm2": np.ascontiguousarray(np.repeat(
                base_full[s0:s0 + s_sh].reshape(n_st, 128).T[:, :, None],
                4, axis=2)),
        }
        in_maps.append(m)
    return in_maps, offconst


def _get_nc(s_sh, offconst):
    key = (s_sh, offconst)
    if key not in _CACHE:
        nc = _build_bass(s_sh, offconst)
        nc.finalize()   # Bacc: runs wait-splitting + register allocation
        _CACHE[key] = nc
    return _CACHE[key]


S_SH = S // 2


def kernel(**inputs) -> np.ndarray:
    from concourse.bass_utils import run_bass_kernel_spmd

    in_maps, offconst = _prep_host(inputs, S_SH)
    nc = _get_nc(S_SH, offconst)
    res = run_bass_kernel_spmd(nc, in_maps, core_ids=list(range(NCORES)))
    y = np.zeros((B, S, D), np.float32)
    n_st = S_SH // 128
    for c in range(NCORES):
        b = c // 2
        hf = c % 2
        yc = res.results[c]["y"]          # [128, n_st, D]
        y[b, hf * S_SH:(hf + 1) * S_SH] = (
            yc.transpose(1, 0, 2).reshape(S_SH, D))
    return y


if __name__ == "__main__":
    import reference
    inputs = {k: np.asarray(v) for k, v in reference.setup_inputs().items()}
    got = kernel(**inputs)
    import jax.numpy as jnp
    exp = np.asarray(reference.reference(**{k: jnp.asarray(v) for k, v in inputs.items()}))
    rel = np.linalg.norm(got - exp) / np.linalg.norm(exp)
    print("Relative error:", rel)
